# revision 8
# baseline (speedup 1.0000x reference)
"""Single-head causal attention kernel for TRN2 (8 NeuronCores, data-parallel).

Problem: x[256,256,384] f32, Wq/Wk/Wv[384,64] f32 ->
  out = softmax(mask((x@Wq)(x@Wk)^T/8)) @ (x@Wv)  [256,256,64] f32

Sharding: batch 256 -> 8 cores x 32 batches. Weights replicated.

Per-core dataflow (all matmuls bf16, fp32 PSUM accumulate):
  1. x loaded with fp32->bf16 cast during SWDGE DMA, [128(T), 384(C)] tiles;
     load groups ramp 2,2,4,4,... so the PE pipeline starts early and
     stays dense (keeps the HAM clock throttle warm)
  2. PE-transpose x -> xT [128(C), T] chunks (projections contract C)
  3. qkv = x @ [Wq|Wk|Wv] via lhsT=xT chunks: natural [T, 192] layout,
     stored into a 193-wide SBUF tile whose col 192 is memset to 1.0 so
     att@v picks up row sums for free (flash-style normalize-after)
  4. PE-transpose q,k -> qT/kT [64(H), 256(T)] (scores contract H)
  5. scores^T[s,t] = sum_h kT[h,s] qT[h,t] -> PSUM, exp(0.125*z) on ACT
  6. causal mask as multiplicative 0/1 tri-tile on DVE (diagonal blocks)
  7. att@v with rhs = qkv[:, s, 128:193] view (v cols + ones col)
  8. one strided DVE reciprocal + one broadcast DVE multiply;
     out stores in 4-batch groups with a 2+2 tail so the last store
     off the critical path is short
"""

import numpy as np

B, T, C, H = 256, 256, 384, 64
NCORES = 8
BPC = B // NCORES  # 32 batches per core
CCH = C // 128  # 3 contraction chunks
TCH = T // 128  # 2 t-chunks

LOADG = [4] * 8  # batches per x-load DMA
STOREG = [4] * 7 + [2, 2]  # batches per out-store DMA

_CACHE = {}


def _build():
    import concourse.bass as bass
    import concourse.mybir as mybir
    import concourse.tile as tile
    from concourse import bacc
    from concourse.bass import broadcast_tensor_aps
    from concourse.masks import make_identity

    fp32 = mybir.dt.float32
    bf16 = mybir.dt.bfloat16

    nc = bacc.Bacc()
    x_d = nc.declare_dram_parameter("x", [BPC, T, C], fp32, isOutput=False)
    wq_d = nc.declare_dram_parameter("wq", [C, H], fp32, isOutput=False)
    wk_d = nc.declare_dram_parameter("wk", [C, H], fp32, isOutput=False)
    wv_d = nc.declare_dram_parameter("wv", [C, H], fp32, isOutput=False)
    out_d = nc.declare_dram_parameter("out", [BPC, T, H], fp32, isOutput=True)

    with tile.TileContext(nc) as tc:
        with (
            tc.tile_pool(name="singles", bufs=1) as singles,
            tc.tile_pool(name="xin", bufs=2) as xin,
            tc.tile_pool(name="work", bufs=3) as work,
            tc.tile_pool(name="vsm", bufs=3) as vsm,
            tc.tile_pool(name="outp", bufs=3) as outp,
            tc.tile_pool(name="ps_xt", bufs=2, space="PSUM") as ps_xt,
            tc.tile_pool(name="ps_qkv", bufs=2, space="PSUM") as ps_qkv,
            tc.tile_pool(name="ps_qkt", bufs=1, space="PSUM") as ps_qkt,
            tc.tile_pool(name="ps_wei", bufs=2, space="PSUM") as ps_wei,
            tc.tile_pool(name="ps_oa", bufs=1, space="PSUM") as ps_oa,
        ):
            # --- constants ---
            ident = singles.tile([128, 128], bf16)
            make_identity(nc, ident)
            # tri[s, t'] = 1.0 if t' >= s else 0  (keep key s for query t'>=s)
            tri = singles.tile([128, 128], bf16)
            nc.gpsimd.memset(tri, 1.0)
            nc.gpsimd.affine_select(
                out=tri, in_=tri,
                compare_op=mybir.AluOpType.is_ge,
                fill=0.0, base=0,
                pattern=[[1, 128]],  # keep where t' - s >= 0, i.e. t' >= s
                channel_multiplier=-1,
            )
            # W packed [128, cch, 192]: cols 0:64=Wq, 64:128=Wk, 128:192=Wv
            # (staged via HWDGE fp32 + gpsimd cast copies: consumers then
            #  see only the single Pool semaphore, and the 4 SWDGE lanes
            #  stay free so each x-load DMA uses a fresh lane)
            wstage = singles.tile([128, CCH, 3 * H], fp32, tag="wstage")
            for wi, wd in enumerate((wq_d, wk_d, wv_d)):
                nc.sync.dma_start(
                    out=wstage[:, :, wi * H:(wi + 1) * H],
                    in_=wd.rearrange("(c p) h -> p c h", p=128),
                )
            wsb = singles.tile([128, CCH, 3 * H], bf16)
            for wi in range(3):
                nc.gpsimd.tensor_copy(
                    wsb[:, :, wi * H:(wi + 1) * H],
                    wstage[:, :, wi * H:(wi + 1) * H])
            # dummy PE op reading the last setup output: makes PE observe the
            # Pool tick past all constants, so per-batch PE instructions never
            # need a second (Pool) wait — PE wait capacity is 1-2 commands
            scratch_ps = ps_qkt.tile([64, 128], bf16, name="scratch",
                                     tag="qkt_ps")
            nc.tensor.transpose(scratch_ps, wsb[:, 0, 0:64], ident)

            xbs = []  # (tile, start_batch, nbatch)
            starts = np.cumsum([0] + LOADG)

            def issue_load(gi):
                n = LOADG[gi]
                s = int(starts[gi])
                xb = xin.tile([128, n, TCH, C], bf16, tag=f"xb{n}")
                nc.gpsimd.dma_start(
                    out=xb,
                    in_=x_d[s:s + n].rearrange("n (c p) f -> p n c f", p=128),
                )
                xbs.append((xb, s, n))

            for gi in range(2):
                issue_load(gi)

            next_load = 2
            cur = 0
            sg_iter = iter(STOREG)
            sg_n = 0
            osb = None
            sg_start = 0
            for b in range(BPC):
                if b >= xbs[cur][1] + xbs[cur][2]:
                    cur += 1
                    if next_load < len(LOADG):
                        issue_load(next_load)
                        next_load += 1
                xb, xs, xn = xbs[cur]
                bi = b - xs
                if sg_n == 0:
                    sg_n = next(sg_iter)
                    sg_start = b
                    osb = outp.tile([128, sg_n, TCH, H], fp32,
                                    tag=f"osb{sg_n}")

                # --- xT via PE transpose (bf16) ---
                xt_ps = ps_xt.tile([128, 2 * CCH, 128], bf16)
                for c in range(CCH):
                    for t in range(TCH):
                        nc.tensor.transpose(
                            xt_ps[:, c * TCH + t, :],
                            xb[:, bi, t, c * 128:(c + 1) * 128],
                            ident,
                        )
                xt = work.tile([128, 2 * CCH, 128], bf16, tag="xt")
                nc.vector.tensor_copy(xt, xt_ps)

                # --- qkv = x @ [Wq|Wk|Wv], natural [T, 192] + ones col ---
                qkv_ps = ps_qkv.tile([128, TCH, 3 * H], fp32)
                for t in range(TCH):
                    for c in range(CCH):
                        nc.tensor.matmul(
                            qkv_ps[:, t, :],
                            lhsT=xt[:, c * TCH + t, :],
                            rhs=wsb[:, c, :],
                            start=(c == 0), stop=(c == CCH - 1),
                        )
                qkv = work.tile([128, TCH, 3 * H + 1], bf16, tag="qkv")
                nc.scalar.copy(qkv[:, :, 0:3 * H], qkv_ps)
                # ones column for row sums in att@v
                nc.gpsimd.memset(qkv[:, :, 3 * H:3 * H + 1], 1.0)

                # --- qT/kT via PE transpose: [64, 2, 256] (q then k) ---
                qkt_ps = ps_qkt.tile([64, 2, T], bf16)
                for qi in range(2):  # 0=q, 1=k
                    for t in range(TCH):
                        nc.tensor.transpose(
                            qkt_ps[:, qi, t * 128:(t + 1) * 128],
                            qkv[:, t, qi * H:(qi + 1) * H],
                            ident,
                        )
                qkt = work.tile([64, 2, T], bf16, tag="qkt")
                nc.vector.tensor_copy(qkt, qkt_ps)

                # --- scores^T: [S, T] ---
                # chunk0: s in 0:128, all t (256); chunk1: s 128:256, t 128:256
                wei_ps = ps_wei.tile([128, 384], fp32)
                nc.tensor.matmul(
                    wei_ps[:, 0:256],
                    lhsT=qkt[:, 1, 0:128], rhs=qkt[:, 0, :],
                    start=True, stop=True,
                )
                nc.tensor.matmul(
                    wei_ps[:, 256:384],
                    lhsT=qkt[:, 1, 128:256], rhs=qkt[:, 0, 128:256],
                    start=True, stop=True,
                )
                # exp(z/8) on ACT, fp32 psum -> bf16 sbuf
                mexp = work.tile([128, 384], bf16, tag="mexp")
                nc.scalar.activation(
                    out=mexp, in_=wei_ps,
                    func=mybir.ActivationFunctionType.Exp,
                    scale=float(H) ** -0.5,
                )
                # causal mask: diagonal blocks only (cols 0:128 & 256:384)
                nc.vector.tensor_mul(mexp[:, 0:128], mexp[:, 0:128], tri)
                nc.vector.tensor_mul(mexp[:, 256:384], mexp[:, 256:384], tri)

                # --- att @ v_aug -> out_aug [T, 65] per t-chunk ---
                # rhs = qkv[:, s, 128:193] view: v cols + ones col
                oa_ps = ps_oa.tile([128, 2, H + 1], fp32)
                nc.tensor.matmul(
                    oa_ps[:, 0, :], lhsT=mexp[:, 0:128],
                    rhs=qkv[:, 0, 2 * H:3 * H + 1],
                    start=True, stop=True,
                )
                nc.tensor.matmul(
                    oa_ps[:, 1, :], lhsT=mexp[:, 128:256],
                    rhs=qkv[:, 0, 2 * H:3 * H + 1],
                    start=True, stop=False,
                )
                nc.tensor.matmul(
                    oa_ps[:, 1, :], lhsT=mexp[:, 256:384],
                    rhs=qkv[:, 1, 2 * H:3 * H + 1],
                    start=False, stop=True,
                )

                # --- normalize (recip + broadcast multiply on DVE) ---
                rec = vsm.tile([128, 2], fp32, tag="rec")
                nc.vector.reciprocal(rec, oa_ps[:, :, H])
                o_ap, r_ap = broadcast_tensor_aps(
                    oa_ps[:, :, 0:H], rec[:, :, None])
                nc.vector.tensor_mul(osb[:, b - sg_start], o_ap, r_ap)

                sg_n -= 1
                if sg_n == 0:
                    n = osb.shape[1]
                    nc.sync.dma_start(
                        out=out_d[sg_start:sg_start + n].rearrange(
                            "n (c p) h -> p n c h", p=128),
                        in_=osb,
                    )
    nc.compile()
    return nc


def _get_nc():
    if "nc" not in _CACHE:
        _CACHE["nc"] = _build()
    return _CACHE["nc"]


def kernel(x, Wq, Wk, Wv):
    from concourse.bass_utils import run_bass_kernel_spmd

    x = np.ascontiguousarray(np.asarray(x, dtype=np.float32))
    Wq = np.ascontiguousarray(np.asarray(Wq, dtype=np.float32))
    Wk = np.ascontiguousarray(np.asarray(Wk, dtype=np.float32))
    Wv = np.ascontiguousarray(np.asarray(Wv, dtype=np.float32))

    nc = _get_nc()
    in_maps = [
        {"x": x[i * BPC:(i + 1) * BPC], "wq": Wq, "wk": Wk, "wv": Wv}
        for i in range(NCORES)
    ]
    res = run_bass_kernel_spmd(nc, in_maps, list(range(NCORES)))
    return np.concatenate([res.results[i]["out"] for i in range(NCORES)], axis=0)


# revision 27
# speedup vs baseline: 1.1913x; 1.1913x over previous
"""Single-head causal attention kernel for TRN2 (8 NeuronCores, data-parallel).

Problem: x[256,256,384] f32, Wq/Wk/Wv[384,64] f32 ->
  out = softmax(mask((x@Wq)(x@Wk)^T/8)) @ (x@Wv)  [256,256,64] f32

Sharding: batch 256 -> 8 cores x 32 batches. Weights replicated.

Per-core dataflow (all matmuls bf16, fp32 PSUM accumulate), software-
pipelined 4 deep so every PE instruction's cross-engine input (DVE/ACT
copies, exp, masks) is produced a full iteration earlier and the PE
never stalls mid-batch:

  iteration i runs:  stage1(i)   x loads, 6 xT transposes, xt copy (DVE),
                                 6 qkv MMs (PSUM banks alternating t0/t1),
                                 qkv copy (ACT) + ones-col memset
                     stageQK(i-1) 4 q/k transposes, qkt copy (DVE)
                     stageSC(i-2) 2 score MMs, exp (ACT), causal masks (DVE)
                     stageAV(i-3) 3 att@v MMs (rhs = v cols + ones col view),
                                 reciprocal + broadcast normalize (DVE),
                                 grouped stores (1-batch groups at the tail)
"""

import numpy as np

B, T, C, H = 256, 256, 384, 64
NCORES = 8
BPC = B // NCORES  # 32 batches per core
CCH = C // 128  # 3 contraction chunks
TCH = T // 128  # 2 t-chunks

LOADG = [2, 2, 4, 4, 4, 4, 4, 4, 4]  # batches per x-load DMA
STOREG = [4] * 7 + [1, 1, 1, 1]  # batches per out-store DMA

_CACHE = {}


def _build():
    import concourse.bass as bass
    import concourse.mybir as mybir
    import concourse.tile as tile
    from concourse import bacc
    from concourse.bass import broadcast_tensor_aps
    from concourse.masks import make_identity

    fp32 = mybir.dt.float32
    bf16 = mybir.dt.bfloat16

    nc = bacc.Bacc()
    x_d = nc.declare_dram_parameter("x", [BPC, T, C], fp32, isOutput=False)
    wq_d = nc.declare_dram_parameter("wq", [C, H], fp32, isOutput=False)
    wk_d = nc.declare_dram_parameter("wk", [C, H], fp32, isOutput=False)
    wv_d = nc.declare_dram_parameter("wv", [C, H], fp32, isOutput=False)
    out_d = nc.declare_dram_parameter("out", [BPC, T, H], fp32, isOutput=True)

    with tile.TileContext(nc) as tc:
        with (
            tc.tile_pool(name="singles", bufs=1) as singles,
            tc.tile_pool(name="xin", bufs=3) as xin,
            tc.tile_pool(name="qkvp", bufs=5) as qkvp,
            tc.tile_pool(name="work", bufs=3) as work,
            tc.tile_pool(name="vsm", bufs=3) as vsm,
            tc.tile_pool(name="outp", bufs=3) as outp,
            tc.tile_pool(name="ps_xt", bufs=2, space="PSUM") as ps_xt,
            tc.tile_pool(name="ps_qkv", bufs=1, space="PSUM") as ps_qkv,
            tc.tile_pool(name="ps_qkt", bufs=1, space="PSUM") as ps_qkt,
            tc.tile_pool(name="ps_wei", bufs=2, space="PSUM") as ps_wei,
            tc.tile_pool(name="ps_oa", bufs=1, space="PSUM") as ps_oa,
        ):
            # --- constants ---
            ident = singles.tile([128, 128], bf16)
            make_identity(nc, ident)

            # issue the first x loads before anything else queues on
            # gpsimd (the W casts below block on the W HWDGE arrival,
            # which would delay the x transfers by ~2us)
            xbs = []  # (tile, start_batch, nbatch)
            starts = np.cumsum([0] + LOADG)

            def issue_load(gi):
                n = LOADG[gi]
                s = int(starts[gi])
                xb = xin.tile([128, n, TCH, C], bf16, name=f"xb_{gi}",
                              tag=f"xb{n}")
                nc.gpsimd.dma_start(
                    out=xb,
                    in_=x_d[s:s + n].rearrange("n (c p) f -> p n c f", p=128),
                )
                xbs.append((xb, s, n))

            for gi in range(3):
                issue_load(gi)

            # tri[s, t'] = 1.0 if t' >= s else 0
            tri = singles.tile([128, 128], bf16)
            nc.gpsimd.memset(tri, 1.0)
            nc.gpsimd.affine_select(
                out=tri, in_=tri,
                compare_op=mybir.AluOpType.is_ge,
                fill=0.0, base=0,
                pattern=[[1, 128]],
                channel_multiplier=-1,
            )
            # W packed [128, cch, 192]: cols 0:64=Wq, 64:128=Wk, 128:192=Wv
            # (staged via HWDGE fp32 + gpsimd cast copies)
            wstage = singles.tile([128, CCH, 3 * H], fp32, tag="wstage")
            for wi, wd in enumerate((wq_d, wk_d, wv_d)):
                nc.sync.dma_start(
                    out=wstage[:, :, wi * H:(wi + 1) * H],
                    in_=wd.rearrange("(c p) h -> p c h", p=128),
                )
            wsb = singles.tile([128, CCH, 3 * H], bf16)
            for wi in range(3):
                nc.gpsimd.tensor_copy(
                    wsb[:, :, wi * H:(wi + 1) * H],
                    wstage[:, :, wi * H:(wi + 1) * H])
            # dummy PE op reading the last setup output: makes PE observe
            # the Pool tick past all constants
            scratch_ps = ps_qkt.tile([64, 2, T], bf16, name="scratch",
                                     tag="qkt_ps")
            nc.tensor.transpose(scratch_ps[:, 0, 0:128], ident[:, 0:64],
                                ident)

            # --- pipeline state ---
            state = {}  # b -> dict(qkv=, qkt=, mexp=)
            ld = {"next": 3, "cur": 0}

            sg_iter = iter(STOREG)
            sg = {"n": 0, "left": 0, "start": 0, "osb": None}

            def stage1(b):
                if b >= xbs[ld["cur"]][1] + xbs[ld["cur"]][2]:
                    ld["cur"] += 1
                    if ld["next"] < len(LOADG):
                        issue_load(ld["next"])
                        ld["next"] += 1
                xb, xs, xn = xbs[ld["cur"]]
                bi = b - xs

                # xT via PE transpose (bf16)
                xt_ps = ps_xt.tile([128, 2 * CCH, 128], bf16)
                for c in range(CCH):
                    for t in range(TCH):
                        nc.tensor.transpose(
                            xt_ps[:, c * TCH + t, :],
                            xb[:, bi, t, c * 128:(c + 1) * 128],
                            ident,
                        )
                xt = work.tile([128, 2 * CCH, 128], bf16, tag="xt")
                nc.vector.tensor_copy(xt, xt_ps)

                # qkv MMs: c-outer/t-inner, t0/t1 accumulators in separate
                # PSUM banks ([128, 2, 512] spans 2 banks) so drains overlap
                qkv_ps = ps_qkv.tile([128, TCH, 512], fp32)
                for c in range(CCH):
                    for t in range(TCH):
                        nc.tensor.matmul(
                            qkv_ps[:, t, 0:3 * H],
                            lhsT=xt[:, c * TCH + t, :],
                            rhs=wsb[:, c, :],
                            start=(c == 0), stop=(c == CCH - 1),
                        )
                qkv = qkvp.tile([128, TCH, 3 * H + 1], bf16, tag="qkv")
                nc.scalar.copy(qkv[:, :, 0:3 * H], qkv_ps[:, :, 0:3 * H])
                nc.gpsimd.memset(qkv[:, :, 3 * H:3 * H + 1], 1.0)
                state[b] = {"qkv": qkv}

            def stage_qkt(b):
                st = state[b]
                qkv = st["qkv"]
                qkt_ps = ps_qkt.tile([64, 2, T], bf16, tag="qkt_ps")
                for qi in range(2):  # 0=q, 1=k
                    for t in range(TCH):
                        nc.tensor.transpose(
                            qkt_ps[:, qi, t * 128:(t + 1) * 128],
                            qkv[:, t, qi * H:(qi + 1) * H],
                            ident,
                        )
                qkt = work.tile([64, 2, T], bf16, tag="qkt")
                nc.vector.tensor_copy(qkt, qkt_ps)
                st["qkt"] = qkt

            def stage_sc(b):
                st = state[b]
                qkt = st["qkt"]
                wei_ps = ps_wei.tile([128, 384], fp32)
                nc.tensor.matmul(
                    wei_ps[:, 0:256],
                    lhsT=qkt[:, 1, 0:128], rhs=qkt[:, 0, :],
                    start=True, stop=True,
                )
                nc.tensor.matmul(
                    wei_ps[:, 256:384],
                    lhsT=qkt[:, 1, 128:256], rhs=qkt[:, 0, 128:256],
                    start=True, stop=True,
                )
                mexp = work.tile([128, 384], bf16, tag="mexp")
                nc.scalar.activation(
                    out=mexp, in_=wei_ps,
                    func=mybir.ActivationFunctionType.Exp,
                    scale=float(H) ** -0.5,
                )
                nc.gpsimd.tensor_mul(mexp[:, 0:128], mexp[:, 0:128], tri)
                nc.gpsimd.tensor_mul(mexp[:, 256:384], mexp[:, 256:384], tri)
                st["mexp"] = mexp

            def stage_av(b):
                st = state.pop(b)
                qkv, mexp = st["qkv"], st["mexp"]
                if sg["left"] == 0:
                    sg["n"] = next(sg_iter)
                    sg["left"] = sg["n"]
                    sg["start"] = b
                    sg["osb"] = outp.tile([128, sg["n"], TCH, H], fp32,
                                          name=f"osb_{b}",
                                          tag=f"osb{sg['n']}")
                osb = sg["osb"]

                oa_ps = ps_oa.tile([128, 2, H + 1], fp32)
                nc.tensor.matmul(
                    oa_ps[:, 0, :], lhsT=mexp[:, 0:128],
                    rhs=qkv[:, 0, 2 * H:3 * H + 1],
                    start=True, stop=True,
                )
                nc.tensor.matmul(
                    oa_ps[:, 1, :], lhsT=mexp[:, 128:256],
                    rhs=qkv[:, 0, 2 * H:3 * H + 1],
                    start=True, stop=False,
                )
                nc.tensor.matmul(
                    oa_ps[:, 1, :], lhsT=mexp[:, 256:384],
                    rhs=qkv[:, 1, 2 * H:3 * H + 1],
                    start=False, stop=True,
                )

                rec = vsm.tile([128, 2], fp32, tag="rec")
                nc.vector.reciprocal(rec, oa_ps[:, :, H])
                o_ap, r_ap = broadcast_tensor_aps(
                    oa_ps[:, :, 0:H], rec[:, :, None])
                nc.vector.tensor_mul(osb[:, b - sg["start"]], o_ap, r_ap)

                sg["left"] -= 1
                if sg["left"] == 0:
                    n = sg["n"]
                    nc.sync.dma_start(
                        out=out_d[sg["start"]:sg["start"] + n].rearrange(
                            "n (c p) h -> p n c h", p=128),
                        in_=osb,
                    )

            for i in range(BPC + 3):
                if i < BPC:
                    stage1(i)
                if 1 <= i < BPC + 1:
                    stage_qkt(i - 1)
                if 2 <= i < BPC + 2:
                    stage_sc(i - 2)
                if 3 <= i:
                    stage_av(i - 3)
    nc.compile()
    return nc


def _get_nc():
    if "nc" not in _CACHE:
        _CACHE["nc"] = _build()
    return _CACHE["nc"]


def kernel(x, Wq, Wk, Wv):
    from concourse.bass_utils import run_bass_kernel_spmd

    x = np.ascontiguousarray(np.asarray(x, dtype=np.float32))
    Wq = np.ascontiguousarray(np.asarray(Wq, dtype=np.float32))
    Wk = np.ascontiguousarray(np.asarray(Wk, dtype=np.float32))
    Wv = np.ascontiguousarray(np.asarray(Wv, dtype=np.float32))

    nc = _get_nc()
    in_maps = [
        {"x": x[i * BPC:(i + 1) * BPC], "wq": Wq, "wk": Wk, "wv": Wv}
        for i in range(NCORES)
    ]
    res = run_bass_kernel_spmd(nc, in_maps, list(range(NCORES)))
    return np.concatenate([res.results[i]["out"] for i in range(NCORES)], axis=0)
